# revision 22
# baseline (speedup 1.0000x reference)
"""AdaAttention Trainium2 kernel — data-parallel over batch across 8 NeuronCores.

Full shapes: h [1024,512], sentinel [1024,512], att_feats [1024,96,2048] -> out [1024,512].
Per core: b=128 batch rows; 24 chunks of 4 slots (512 tokens).

Host pre-casts att_feats to fp8 AND pre-transposes it into the exact MM1
moving layout (attf[c, p, gt, s, b] u16 byte-pairs, f = gt*256+2p+{0,1}), so
the device reads 25MB/core contiguously. PE continuously fed -> max p-state.

out = tanh((cHat + h) @ W_oT + b_o), cHat = sum_s alpha_s * img_all_s,
img_all = [sentinel, att],  att = relu(att_feats @ W_aeT + b_ae).
C = sum_s exp(l_s) * img_all_s accumulates in one persistent PSUM bank
(no max subtraction: |logits| <~ 12, exp safe in f32); divide by d at end.

Per chunk (all matmul paths fp8 where precision allows):
  MM1 (DoubleRow fp8, W_ae x64): ps1 = 64*att_raw
    -> attT_bf[r, x] = relu(ps1/64 + b_ae)      bf16 (ACT; exact att)
    -> attT8[r, x]  = max(ps1 + 64*b_ae, 0)     fp8 = 64*att (DVE 2-op)
  MM2 (DoubleRow fp8, attT8 stationary, W_c x64 moving): ps2 = 4096*attE
  hA = tanh((ps2 + he_c4096)/4096) [DVE add + ACT tanh scale]
  logit col = mult+reduce(hA * wal_rep) on GPSIMD
  transposes (PE): att_nat[b, slot, r] = attT_bf^T  (exact bf16 att)
  flash (deferred 1 chunk): C += diag(e_t) @ att_nat_t
Final: pre = C/d + h; out = tanh(pre @ W_oT + b_o) via PE transposes + matmul.
b_al skipped (softmax shift-invariant).
"""
import sys

for p in ("/opt/trn_rl_repo", "/opt/pypackages"):
    if p not in sys.path:
        sys.path.insert(0, p)

import numpy as np
import ml_dtypes
from contextlib import ExitStack

import concourse.bass as bass
import concourse.bacc as bacc
import concourse.mybir as mybir
from concourse import tile

F32 = mybir.dt.float32
BF16 = mybir.dt.bfloat16
FP8 = mybir.dt.float8e4
U16 = mybir.dt.uint16
AF = mybir.ActivationFunctionType
ALU = mybir.AluOpType
DR = mybir.MatmulPerfMode.DoubleRow

NCORES = 8
B_LOC = 128          # batch rows per core
S = 96               # attention slots
F = 2048             # att feature size
R = 512              # rnn size
A = 512              # att hidden size
XCHUNK = 512         # tokens per pipeline chunk (4 s-tiles)
NCHUNKS = (B_LOC * S) // XCHUNK   # 24
S_PER_CHUNK = XCHUNK // B_LOC     # 4
GT = F // 256        # 8 double-row f-tiles (256 f's each)
RT = R // 128        # 4
RBLK = R // 256      # 2 double-row r-tiles for MM2
AT = A // 128        # 4
WSCALE = 64.0        # fp8 weight scale for W_ae (and W_c); 64*att < 448 fp8 max
SC2 = WSCALE * WSCALE  # 4096: scale of MM2 psum


def build_nc():
    nc = bacc.Bacc("TRN2", target_bir_lowering=False, debug=False)

    # ---- DRAM parameters (per-core shard shapes) ----
    # attf[c, p, gt, s, b]: u16 = fp8 byte pair (f = gt*256 + 2p + {0,1}),
    # token = (b, slot c*4+s) — host-prepared MM1 moving layout.
    attf_d = nc.declare_dram_parameter("attf", [NCHUNKS, 128, GT, S_PER_CHUNK, 128],
                                       U16, isOutput=False)
    h_in = nc.declare_dram_parameter("h", [B_LOC, R], F32, isOutput=False)
    sent_in = nc.declare_dram_parameter("sentinel", [B_LOC, R], F32, isOutput=False)
    # w_ae_dr[p, gt, i, r] = (W_ae*64).T[f, r], f = gt*256 + 2p + i   (fp8)
    w_ae_d = nc.declare_dram_parameter("w_ae_dr", [128, GT, 2, R], FP8, isOutput=False)
    # w_c8[p, rblk, a, two] = (W_c*64).T[r, a], r = rblk*256 + two*128 + p (fp8)
    w_c8_d = nc.declare_dram_parameter("w_c8", [128, RBLK, A, 2], FP8, isOutput=False)
    w_s_t = nc.declare_dram_parameter("w_s_t", [128, RT, A], BF16, isOutput=False)   # x4096
    w_h_t = nc.declare_dram_parameter("w_h_t", [128, RT, A], BF16, isOutput=False)   # x4096
    w_o_t = nc.declare_dram_parameter("w_o_t", [128, RT, R], BF16, isOutput=False)
    wal_rep_d = nc.declare_dram_parameter("wal_rep", [128, A], BF16, isOutput=False)
    b_ae_d = nc.declare_dram_parameter("b_ae", [128, RT], F32, isOutput=False)
    bcbh_d = nc.declare_dram_parameter("bcbh", [128, A], F32, isOutput=False)   # (b_c+b_h)*4096
    bsbh_d = nc.declare_dram_parameter("bsbh", [128, A], F32, isOutput=False)   # (b_s+b_h)*4096
    b_o_bc_d = nc.declare_dram_parameter("b_o_bcast", [128, R], F32, isOutput=False)
    ident_d = nc.declare_dram_parameter("ident", [128, 128], BF16, isOutput=False)
    ident4_d = nc.declare_dram_parameter("ident4", [128, S_PER_CHUNK, 128], BF16, isOutput=False)
    out_d = nc.declare_dram_parameter("out", [B_LOC, R], F32, isOutput=True)

    with tile.TileContext(nc) as tc, ExitStack() as ctx:
        # ---- pools ----
        cp = ctx.enter_context(tc.tile_pool(name="consts", bufs=1))
        attf_p = ctx.enter_context(tc.tile_pool(name="attf", bufs=4))
        attT_p = ctx.enter_context(tc.tile_pool(name="attT", bufs=3))
        attT8_p = ctx.enter_context(tc.tile_pool(name="attT8", bufs=3))
        anat_p = ctx.enter_context(tc.tile_pool(name="anat", bufs=3))
        hat_p = ctx.enter_context(tc.tile_pool(name="hat", bufs=6))
        small_p = ctx.enter_context(tc.tile_pool(name="small", bufs=6))
        msel_p = ctx.enter_context(tc.tile_pool(name="msel", bufs=4))
        lcol_p = ctx.enter_context(tc.tile_pool(name="lcol", bufs=3))
        soft_p = ctx.enter_context(tc.tile_pool(name="soft", bufs=1))
        ps_mm1 = ctx.enter_context(tc.tile_pool(name="ps_mm1", bufs=2, space="PSUM"))
        ps_mm2 = ctx.enter_context(tc.tile_pool(name="ps_mm2", bufs=4, space="PSUM"))
        ps_tr = ctx.enter_context(tc.tile_pool(name="ps_tr", bufs=1, space="PSUM"))
        ps_chat = ctx.enter_context(tc.tile_pool(name="ps_chat", bufs=1, space="PSUM"))

        attf_tiles = {}

        def stage_load(c):
            attf = attf_p.tile([128, GT, S_PER_CHUNK, 128], U16, tag="attf",
                               name=f"attf_{c}")
            nc.scalar.dma_start(out=attf[:, 0:GT // 2], in_=attf_d[c, :, 0:GT // 2])
            nc.sync.dma_start(out=attf[:, GT // 2:], in_=attf_d[c, :, GT // 2:])
            attf_tiles[c] = attf

        def const_tile(name, shape, dtype, src):
            t = cp.tile(shape, dtype, tag=name, name=name)
            nc.gpsimd.dma_start(out=t[:], in_=src[:])
            return t

        # chunk 0 feed first; mm1 consts next, then prep deps, then b1 deps
        stage_load(0)
        w_ae = const_tile("w_ae", [128, GT, 2, R], FP8, w_ae_d)
        b_ae = const_tile("b_ae", [128, RT], F32, b_ae_d)
        ident = const_tile("ident", [128, 128], BF16, ident_d)
        h_bf = cp.tile([B_LOC, R], BF16, tag="h_bf", name="h_bf")
        nc.gpsimd.dma_start(out=h_bf[:], in_=h_in[:])
        sent_bf = cp.tile([B_LOC, R], BF16, tag="sent_bf", name="sent_bf")
        nc.gpsimd.dma_start(out=sent_bf[:], in_=sent_in[:])
        w_h = const_tile("w_h", [128, RT, A], BF16, w_h_t)
        w_s = const_tile("w_s", [128, RT, A], BF16, w_s_t)
        w_o = const_tile("w_o", [128, RT, R], BF16, w_o_t)
        wal_rep = const_tile("wal_rep", [128, A], BF16, wal_rep_d)
        bcbh = const_tile("bcbh", [128, A], F32, bcbh_d)
        bsbh = const_tile("bsbh", [128, A], F32, bsbh_d)
        b_o_bc = const_tile("b_o_bc", [128, R], F32, b_o_bc_d)
        w_c8 = const_tile("w_c8", [128, RBLK, A, 2], FP8, w_c8_d)
        ident4 = const_tile("ident4", [128, S_PER_CHUNK, 128], BF16, ident4_d)

        stage_load(1)

        # e_sb[:, t] = exp(logit_t), t=0 sentinel, t=1.. att slots
        e_sb = cp.tile([B_LOC, 1 + S], F32, tag="e_sb", name="e_sb")
        prep_out = {}

        def prep_a():
            """PE-heavy prep + sentinel logit chain (no PE work after the chain)."""
            hT = cp.tile([128, RT, B_LOC], BF16, tag="hT", name="hT")
            sentT = cp.tile([128, RT, B_LOC], BF16, tag="sentT", name="sentT")
            for rb in range(RT):
                pt = ps_mm2.tile([128, 1024], BF16, tag="mm2", name=f"pt_h{rb}")
                nc.tensor.transpose(pt[:, :128], h_bf[:, rb * 128:(rb + 1) * 128], ident[:])
                nc.vector.tensor_copy(hT[:, rb, :], pt[:, :128])
                pt2 = ps_mm2.tile([128, 1024], BF16, tag="mm2", name=f"pt_s{rb}")
                nc.tensor.transpose(pt2[:, :128], sent_bf[:, rb * 128:(rb + 1) * 128], ident[:])
                nc.vector.tensor_copy(sentT[:, rb, :], pt2[:, :128])

            # h_e*4096 (w_h host-scaled) -> he_c = 4096*(h_e + b_c + b_h)
            ps_he = ps_mm2.tile([128, A], F32, tag="mm2", name="ps_he")
            for rb in range(RT):
                nc.tensor.matmul(ps_he[:], hT[:, rb, :], w_h[:, rb, :],
                                 start=(rb == 0), stop=(rb == RT - 1))
            he_c = cp.tile([B_LOC, A], F32, tag="he_c", name="he_c")
            nc.vector.tensor_tensor(out=he_c[:], in0=ps_he[:], in1=bcbh[:], op=ALU.add)
            he_s = cp.tile([B_LOC, A], F32, tag="he_s", name="he_s")
            nc.vector.tensor_tensor(out=he_s[:], in0=ps_he[:], in1=bsbh[:], op=ALU.add)

            # sentinel embed (x4096)
            ps_se = ps_mm2.tile([128, A], F32, tag="mm2", name="ps_se")
            for rb in range(RT):
                nc.tensor.matmul(ps_se[:], sentT[:, rb, :], w_s[:, rb, :],
                                 start=(rb == 0), stop=(rb == RT - 1))

            # sentinel logit chain (DVE/ACT only)
            pre0 = small_p.tile([B_LOC, A], BF16, tag="hatmp", name="pre0")
            nc.vector.tensor_tensor(out=pre0[:], in0=ps_se[:], in1=he_s[:], op=ALU.add)
            hA0 = hat_p.tile([B_LOC, A], BF16, tag="hat", name="hA0")
            nc.scalar.activation(hA0[:], pre0[:], AF.Tanh, scale=1.0 / SC2)
            ttr0 = small_p.tile([B_LOC, A], BF16, tag="ttro", name="ttr0")
            lc0 = lcol_p.tile([B_LOC, 1], F32, tag="lc", name="lc0")
            nc.vector.tensor_tensor(out=ttr0[:], in0=hA0[:], in1=wal_rep[:], op=ALU.mult)
            nc.vector.tensor_reduce(out=lc0[:], in_=ttr0[:], op=ALU.add,
                                    axis=mybir.AxisListType.X)
            nc.scalar.activation(e_sb[:, 0:1], lc0[:], AF.Exp)
            prep_out.update(he_c=he_c)

        def prep_b():
            """Open the persistent cHat accumulation with the sentinel term."""
            ps_cH = ps_chat.tile([B_LOC, R], F32, name="ps_cH")
            ms0 = msel_p.tile([128, 128], BF16, tag="msel", name="ms0")
            nc.vector.tensor_scalar(out=ms0[:], in0=ident[:], scalar1=e_sb[:, 0:1],
                                    scalar2=None, op0=ALU.mult)
            nc.tensor.matmul(ps_cH[:], ms0[:], sent_bf[:], start=True,
                             stop=False, skip_group_check=True)
            prep_out.update(ps_cH=ps_cH)

        # ---- main pipeline stages ----
        attT_chunks = {}
        anat_chunks = {}

        def stage_mm1(c):
            attf = attf_tiles.pop(c)
            attT = attT_p.tile([128, RT, XCHUNK], BF16, tag="attT", name=f"attT_{c}")
            attT8 = attT8_p.tile([128, RT, XCHUNK], FP8, tag="attT8", name=f"attT8_{c}")
            for rb in range(RT):
                ps1 = ps_mm1.tile([128, XCHUNK], F32, tag="mm1", name=f"ps1_{c}_{rb}")
                for gt in range(GT):
                    rhs = attf[:, gt].bitcast(FP8).rearrange("p s (n two) -> p two s n", two=2)
                    nc.tensor.matmul(ps1[:], w_ae[:, gt, :, rb * 128:(rb + 1) * 128],
                                     rhs, start=(gt == 0), stop=(gt == GT - 1),
                                     perf_mode=DR)
                # exact-scale bf16 att (for xbars/flash)
                nc.scalar.activation(attT[:, rb, :], ps1[:], AF.Relu,
                                     bias=b_ae[:, rb:rb + 1], scale=1.0 / WSCALE)
                # fp8 att copy via SWDGE cast DMA on the idle gpsimd queue
                # (MM2 stationary; W_c host-scaled x4096)
                nc.gpsimd.dma_start(out=attT8[:, rb, :], in_=attT[:, rb, :])
            attT_chunks[c] = (attT, attT8)

        def stage_b1(c):
            attT, attT8 = attT_chunks.pop(c)
            he_c = prep_out["he_c"]
            # anat[tok_p, rb, i, rr] = att[r=rb*128+rr, tok=i*128+p]
            anat = anat_p.tile([128, RT, S_PER_CHUNK, 128], BF16, tag="anat",
                               name=f"anat_{c}")
            lcol = lcol_p.tile([B_LOC, S_PER_CHUNK], F32, tag="lc", name=f"lcol_{c}")
            # transposes on the sync-queue DMA xbar (PE freed; bf16 = u16 elems)
            for rb in range(RT):
                nc.sync.dma_start(out=anat[:, rb].bitcast(U16),
                                  in_=attT[:, rb, :].bitcast(U16), transpose=True)
            ps2s = []
            for i in range(S_PER_CHUNK):
                ps2 = ps_mm2.tile([128, A], F32, tag="mm2", name=f"ps2_{c}_{i}")
                for rblk in range(RBLK):
                    mov = w_c8[:, rblk].rearrange("p a two -> p two a")
                    nc.tensor.matmul(ps2[:], attT8[:, 2 * rblk:2 * rblk + 2,
                                                   i * 128:(i + 1) * 128],
                                     mov, start=(rblk == 0), stop=(rblk == RBLK - 1),
                                     perf_mode=DR)
                ps2s.append(ps2)
            # DVE adds chase the MM2 groups; ACT tanh chases the adds
            tmps = []
            for i in range(S_PER_CHUNK):
                tmp = small_p.tile([B_LOC, A], BF16, tag="hatmp", name=f"hatmp_{c}_{i}")
                nc.vector.tensor_tensor(out=tmp[:], in0=ps2s[i][:], in1=he_c[:], op=ALU.add)
                tmps.append(tmp)
            hts = []
            for i in range(S_PER_CHUNK):
                ht = hat_p.tile([B_LOC, A], BF16, tag="hat", name=f"hat_{c}_{i}")
                nc.scalar.activation(ht[:], tmps[i][:], AF.Tanh, scale=1.0 / SC2)
                hts.append(ht)
            # logits: DVE mult + reduce per slot, then one exp (ACT)
            # (tensor_tensor_reduce hangs on HW — verified; keep ops split)
            for i in range(S_PER_CHUNK):
                ttro = small_p.tile([B_LOC, A], BF16, tag="ttro", name=f"ttro_{c}_{i}")
                nc.vector.tensor_tensor(out=ttro[:], in0=hts[i][:], in1=wal_rep[:],
                                        op=ALU.mult)
                nc.vector.tensor_reduce(out=lcol[:, i:i + 1], in_=ttro[:], op=ALU.add,
                                        axis=mybir.AxisListType.X)
            anat_chunks[c] = anat
            nc.scalar.activation(
                e_sb[:, 1 + c * S_PER_CHUNK: 1 + (c + 1) * S_PER_CHUNK], lcol[:], AF.Exp)

        def stage_flash(c):
            ps_cH = prep_out["ps_cH"]
            anat = anat_chunks.pop(c)
            ms4 = msel_p.tile([128, S_PER_CHUNK, 128], BF16, tag="msel", name=f"ms4_{c}")
            nc.vector.tensor_tensor(
                out=ms4[:], in0=ident4[:],
                in1=e_sb[:, 1 + c * S_PER_CHUNK: 1 + (c + 1) * S_PER_CHUNK]
                    .unsqueeze(2).broadcast_to([128, S_PER_CHUNK, 128]),
                op=ALU.mult)
            for i in range(S_PER_CHUNK):
                t = c * S_PER_CHUNK + i
                nc.tensor.matmul(ps_cH[:], ms4[:, i, :], anat[:, :, i, :],
                                 start=False, stop=(t == S - 1), skip_group_check=True)

        # ---- build pipeline ----
        stage_load(2)
        stage_mm1(0)
        prep_a()
        stage_load(3)
        stage_mm1(1)
        prep_b()
        for c in range(2, NCHUNKS + 5):
            if c + 2 < NCHUNKS:
                stage_load(c + 2)
            if 3 <= c <= NCHUNKS + 2:
                stage_flash(c - 3)
            if 2 <= c <= NCHUNKS + 1:
                stage_b1(c - 2)
            if c < NCHUNKS:
                stage_mm1(c)

        # ---- final: out = tanh((C/d + h) @ W_oT + b_o) ----
        ps_cH = prep_out["ps_cH"]
        dsum = soft_p.tile([B_LOC, 1], F32, tag="soft", name="dsum")
        nc.vector.tensor_reduce(out=dsum[:], in_=e_sb[:], op=ALU.add,
                                axis=mybir.AxisListType.X)
        rin = soft_p.tile([B_LOC, 1], F32, tag="rin", name="rin")
        nc.vector.reciprocal(rin[:], dsum[:])
        chn = soft_p.tile([B_LOC, R], F32, tag="chn", name="chn")
        nc.vector.tensor_scalar(out=chn[:], in0=ps_cH[:], scalar1=rin[:],
                                scalar2=None, op0=ALU.mult)
        preb = soft_p.tile([B_LOC, R], BF16, tag="preb", name="preb")
        nc.vector.tensor_tensor(out=preb[:], in0=chn[:], in1=h_bf[:], op=ALU.add)
        preT = soft_p.tile([128, RT, B_LOC], BF16, tag="preT", name="preT")
        ptf = ps_tr.tile([128, 2 * RT, 128], BF16, tag="tr", name="ptf")
        for rb in range(RT):
            nc.tensor.transpose(ptf[:, rb, :], preb[:, rb * 128:(rb + 1) * 128], ident[:])
        nc.vector.tensor_copy(preT[:], ptf[:, 0:RT, :])
        ps_out = ps_mm2.tile([128, R], F32, tag="mm2", name="ps_out")
        for rb in range(RT):
            nc.tensor.matmul(ps_out[:], preT[:, rb, :], w_o[:, rb, :],
                             start=(rb == 0), stop=(rb == RT - 1))
        preo = soft_p.tile([B_LOC, R], F32, tag="preo", name="preo")
        nc.vector.tensor_tensor(out=preo[:], in0=ps_out[:], in1=b_o_bc[:], op=ALU.add)
        out_sb = soft_p.tile([B_LOC, R], F32, tag="out_sb", name="out_sb")
        nc.scalar.activation(out_sb[:], preo[:], AF.Tanh)
        nc.sync.dma_start(out=out_d[:], in_=out_sb[:])

    nc.compile()
    return nc


# ---------------- host side ----------------
_NC_CACHE = None


def _get_nc():
    global _NC_CACHE
    if _NC_CACHE is None:
        _NC_CACHE = build_nc()
    return _NC_CACHE


def prep_shared(W_ae, b_ae, W_c, b_c, W_s, b_s, W_h, b_h, W_al, b_al, W_o, b_o):
    bf = ml_dtypes.bfloat16
    f8 = ml_dtypes.float8_e4m3

    def wt(w, nt, scale=1.0):  # [p, t, n] = w.T[128*t + p, n] * scale
        wT = np.ascontiguousarray(np.asarray(w, np.float32).T * scale)
        return np.ascontiguousarray(
            wT.reshape(nt, 128, wT.shape[1]).transpose(1, 0, 2)).astype(bf)

    def bt(b, nt, scale=1.0):  # [p, t] = b[128*t + p] * scale
        return np.ascontiguousarray(
            np.asarray(b, np.float32).reshape(nt, 128).T * scale).astype(np.float32)

    def rep(v):  # [128, len(v)] f32 replicated rows
        return np.ascontiguousarray(
            np.tile(np.asarray(v, np.float32)[None, :], (128, 1)))

    # w_ae_dr[p, gt, i, r] = (W_ae*WSCALE).T[f, r], f = gt*256 + 2p + i
    waeT = (np.asarray(W_ae, np.float32) * WSCALE).T.astype(f8)  # [F, R]
    w_ae_dr = np.ascontiguousarray(
        waeT.reshape(GT, 128, 2, R).transpose(1, 0, 2, 3))

    # w_c8[p, rblk, a, two] = (W_c*SC2).T[r, a], r = rblk*256 + two*128 + p
    wcT = (np.asarray(W_c, np.float32) * SC2).T.astype(f8)  # [R, A]
    w_c8 = np.ascontiguousarray(
        wcT.reshape(RBLK, 2, 128, A).transpose(2, 0, 3, 1))

    return {
        "w_ae_dr": w_ae_dr,
        "w_c8": w_c8,
        "w_s_t": wt(W_s, RT, SC2),
        "w_h_t": wt(W_h, RT, SC2),
        "w_o_t": wt(W_o, RT),
        "wal_rep": rep(np.asarray(W_al, np.float32)[0]).astype(bf),
        "b_ae": bt(b_ae, RT),
        "bcbh": rep((np.asarray(b_c, np.float32) + np.asarray(b_h, np.float32)) * SC2),
        "bsbh": rep((np.asarray(b_s, np.float32) + np.asarray(b_h, np.float32)) * SC2),
        "b_o_bcast": rep(b_o),
        "ident": np.eye(128, dtype=bf),
        "ident4": np.ascontiguousarray(
            np.broadcast_to(np.eye(128, dtype=bf)[:, None, :],
                            (128, S_PER_CHUNK, 128))),
    }


def prep_attf(att_feats):
    """fp8-cast + pre-transpose att_feats into the MM1 moving layout.

    Returns [NCORES, NCHUNKS, 128, GT, S_PER_CHUNK, 128] uint16 where
    attf[n, c, p, gt, s, b] packs fp8 bytes f = gt*256 + 2p + {0,1} of
    token (batch row n*128+b, slot c*4+s).
    """
    f8 = ml_dtypes.float8_e4m3
    x = np.asarray(att_feats, np.float32).astype(f8).view(np.uint8)
    # [B, S, F] -> [n, b, c, s, gt, p, two] -> [n, c, p, gt, s, b, two]
    x = x.reshape(NCORES, B_LOC, NCHUNKS, S_PER_CHUNK, GT, 128, 2)
    x = np.ascontiguousarray(x.transpose(0, 2, 5, 4, 3, 1, 6))
    return x.view(np.uint16)[..., 0]


def make_in_maps(h, sentinel, att_feats, shared):
    h = np.asarray(h, np.float32)
    sentinel = np.asarray(sentinel, np.float32)
    attf = prep_attf(att_feats)
    in_maps = []
    for i in range(NCORES):
        sl = slice(i * B_LOC, (i + 1) * B_LOC)
        m = dict(shared)
        m["h"] = np.ascontiguousarray(h[sl])
        m["sentinel"] = np.ascontiguousarray(sentinel[sl])
        m["attf"] = attf[i]
        in_maps.append(m)
    return in_maps


def kernel(h, sentinel, att_feats, W_ae, b_ae, W_c, b_c, W_s, b_s,
           W_h, b_h, W_al, b_al, W_o, b_o):
    shared = prep_shared(W_ae, b_ae, W_c, b_c, W_s, b_s, W_h, b_h, W_al, b_al, W_o, b_o)
    in_maps = make_in_maps(h, sentinel, att_feats, shared)
    nc = _get_nc()
    from concourse.bass_utils import run_bass_kernel_spmd
    res = run_bass_kernel_spmd(nc, in_maps, core_ids=list(range(NCORES)))
    out = np.concatenate([res.results[i]["out"] for i in range(NCORES)], axis=0)
    return np.ascontiguousarray(out.astype(np.float32))


if __name__ == "__main__":
    build_nc()
    print("built ok")
